# revision 7
# baseline (speedup 1.0000x reference)
"""CosFace loss (B=1024, D=512, C=100000) on 8 Trainium2 NeuronCores.

Strategy (tensor-parallel classification head, per sharding hint):
  - Classes sharded 12500/core (padded to 12544 = 98*128 with zero cols;
    pad cols give cos=0 -> exp(-64) ~ 0, harmless).
  - Host prep: weight rows L2-normalized and cast to fp8(e4m3); x rows
    scaled by S/|x| (folds the CosFace scale AND the x-normalization into
    the matmul operand) and cast to fp8; both transposed so the
    contraction dim D is on partitions.  The label-column fixup terms
    (exact S*cos_label and exp deltas) are tiny [B] vectors computed on
    host in f64.
  - Device per core: stream wT fp8 chunks, DoubleRow fp8 matmuls
    (contract 256/instr at 2x rate) accumulate logits [128b, 2048c] in
    PSUM; fused ScalarE pass computes exp(logit - 64) in place with
    accum_out giving per-row partial sum-of-exp (the fixed shift 64 = S
    bounds |logits|, so no running max is needed).
  - One 4KB AllReduce of the [1024] partial sums.
  - Margin fixup: sum gets exp(S*cos_l) swapped for exp(S*cos_l - S*M)
    by subtracting host-computed dneg; logz = 64 + ln(sum), and
    loss = mean(logz - (S*cos_l - S*M)).  Core 0's output is used.
"""

import numpy as np
import ml_dtypes

import concourse.bass as bass
import concourse.mybir as mybir
import concourse.tile as tile
from concourse import bacc
from concourse.bass_utils import run_bass_kernel_spmd

B, D, C = 1024, 512, 100000
S, MARGIN = 64.0, 0.35
SHIFT = 64.0
NCORES = 8
CSHARD = C // NCORES          # 12500 real classes per core
CLOC = 12544                  # padded (98 * 128)
KT = D // 128                 # 4 contraction tiles
MT = B // 128                 # 8 batch tiles
import os as _os
CHUNK = int(_os.environ.get('KCHUNK', '2048'))   # classes per PSUM group
NCHUNK = (CLOC + CHUNK - 1) // CHUNK
WBUFS = int(_os.environ.get('WBUFS', '2'))
PBUFS = int(_os.environ.get('PBUFS', '2'))
# columns [0, VSPLIT) of each chunk take the exact ScalarE exp; columns
# [VSPLIT, CHUNK) take the DVE fast-exp (Schraudolph bf16-bits) path,
# balancing the two engines (ACT 0.83 ns/col vs DVE 1.56 ns/col).
VSPLIT = int(_os.environ.get('VSPLIT', '1280'))
# fast-exp: exp(g - 64) ~ bf16_from_bits(u16(RNE(EXP_A*g + EXP_B)));
# EXP_C tunes the piecewise-linear bias to zero mean error over the
# exp-weighted logit distribution (fitted numerically).
EXP_A = 128.0 / float(np.log(2.0))
EXP_C = -0.057978853647583345
EXP_B = 16256.0 - 64.0 * EXP_A + 128.0 * EXP_C

F32 = mybir.dt.float32
F8 = mybir.dt.float8e4
U16 = mybir.dt.uint16
BF16 = mybir.dt.bfloat16
AF = mybir.ActivationFunctionType
AX = mybir.AxisListType
ALU = mybir.AluOpType
DR = mybir.MatmulPerfMode.DoubleRow

_NC = None
LAST_RESULTS = None
ABLATE = _os.environ.get('ABLATE', 'full')  # full | noexp | nomm | nov | nos
COLLECTIVE = _os.environ.get('COLLECTIVE', '1') == '1'


def _body(nc, tc, xt, wt, scl_in, dneg_in, loss, collective=True):
    from contextlib import ExitStack
    with ExitStack() as ctx:
        singles = ctx.enter_context(tc.tile_pool(name="singles", bufs=1))
        wpool = ctx.enter_context(tc.tile_pool(name="wpool", bufs=WBUFS))
        psump = ctx.enter_context(tc.tile_pool(name="psump", bufs=PBUFS, space="PSUM"))
        dram = ctx.enter_context(tc.tile_pool(name="dram", bufs=1, space="DRAM"))
        # ---- resident inputs ----
        xt_sb = singles.tile([128, KT, B], F8)
        nc.sync.dma_start(out=xt_sb[:, :, :],
                          in_=xt.ap().rearrange("(k p) b -> p k b", p=128))
        scl_sb = singles.tile([128, MT], F32)
        nc.sync.dma_start(out=scl_sb[:, :], in_=scl_in.ap())
        dneg_sb = singles.tile([128, MT], F32)
        nc.sync.dma_start(out=dneg_sb[:, :], in_=dneg_in.ap())

        # const bias columns (activation bias must be a [P,1] AP)
        def const_col(val):
            t = singles.tile([128, 1], F32)
            nc.vector.memset(t[:, :], val)
            return t

        cb_m64 = const_col(-SHIFT)
        cb_fin = const_col(SHIFT + S * MARGIN - 66.0 * float(np.log(2.0)))

        # ---- main loop: logit matmuls + fused exp/accumulate ----
        # sums[:, :, c] <- ScalarE exact-exp partials; sums[:, :, NCHUNK+c]
        # <- DVE fast-exp partials.  One reduce at the end covers both.
        sums = singles.tile([128, MT, 2 * NCHUNK], F32)
        nc.vector.memset(sums[:, :, :], 0.0)
        vpool = ctx.enter_context(tc.tile_pool(name="vpool", bufs=2))
        wt_v = wt.ap().rearrange("(k p) c -> p k c", p=128)
        for c in range(NCHUNK):
            c0 = c * CHUNK
            ncls = min(CHUNK, CLOC - c0)
            scols = min(VSPLIT, ncls)
            vcols = ncls - scols
            wt_c = wpool.tile([128, KT, CHUNK], F8, tag="wt")
            nc.sync.dma_start(out=wt_c[:, :, :ncls], in_=wt_v[:, :, c0:c0 + ncls])
            for m in range(MT):
                g = psump.tile([128, CHUNK], F32, tag="g")
                if ABLATE != "nomm":
                    for g2 in range(2):   # pairs of contraction k-tiles
                        lhsT = xt_sb[:, 2 * g2:2 * g2 + 2, m * 128:(m + 1) * 128]
                        for n in range(0, ncls, 512):
                            nsz = min(512, ncls - n)
                            nc.tensor.matmul(g[:, n:n + nsz], lhsT,
                                             wt_c[:, 2 * g2:2 * g2 + 2, n:n + nsz],
                                             start=(g2 == 0), stop=(g2 == 1),
                                             perf_mode=DR)
                if ABLATE not in ("noexp", "nos"):
                    # in-place on PSUM: we only need accum_out; ScalarE is
                    # closest to PSUM and this avoids an SBUF scratch write
                    nc.scalar.activation(g[:, :scols], g[:, :scols], AF.Exp,
                                         bias=cb_m64[:, :],
                                         accum_out=sums[:, m, c:c + 1])
                if ABLATE not in ("noexp", "nov"):
                    if vcols > 0:
                        y16 = vpool.tile([128, CHUNK - VSPLIT], U16, tag="y")
                        nc.vector.tensor_scalar(
                            y16[:, :vcols], g[:, scols:ncls],
                            EXP_A, EXP_B, ALU.mult, ALU.add)
                        nc.vector.tensor_reduce(
                            sums[:, m, NCHUNK + c:NCHUNK + c + 1],
                            y16[:, :vcols].bitcast(BF16),
                            axis=AX.X, op=ALU.add)

        # ---- reduce partials + AllReduce ----
        se_part = singles.tile([128, MT], F32)
        nc.vector.tensor_reduce(se_part[:, :], sums[:, :, :], axis=AX.X, op=ALU.add)
        full_se = singles.tile([128, MT], F32)
        if collective:
            ar_in = dram.tile([128, MT], F32)
            ar_out = dram.tile([128, MT], F32, addr_space="Shared")
            nc.sync.dma_start(out=ar_in[:, :], in_=se_part[:, :])
            nc.gpsimd.collective_compute(
                "AllReduce", ALU.add,
                replica_groups=[list(range(NCORES))],
                ins=[ar_in.opt()], outs=[ar_out.opt()])
            nc.sync.dma_start(out=full_se[:, :], in_=ar_out[:, :])
        else:
            nc.vector.tensor_scalar_mul(full_se[:, :], se_part[:, :], float(NCORES))

        # ---- logz and loss ----
        adj = singles.tile([128, MT], F32)
        nc.vector.tensor_sub(adj[:, :], full_se[:, :], dneg_sb[:, :])
        ln_adj = singles.tile([128, MT], F32)
        # ACT Ln is inaccurate for tiny args (~1e-21); prescale into [0.1, 10]
        # via the free affine input (ln(adj*2^66) = ln(adj) + 66*ln2, the
        # constant is folded into the final bias below).
        nc.scalar.activation(ln_adj[:, :], adj[:, :], AF.Ln, scale=float(2.0 ** 66))
        lossv = singles.tile([128, MT], F32)
        nc.vector.tensor_sub(lossv[:, :], ln_adj[:, :], scl_sb[:, :])
        rowsum = singles.tile([128, 1], F32)
        junk2 = singles.tile([128, MT], F32)
        nc.scalar.activation(junk2[:, :], lossv[:, :], AF.Identity,
                             accum_out=rowsum[:, :])
        # partition-axis reduce on PE: [1,1] = ones.T @ rowsum
        ones_col = singles.tile([128, 1], F32)
        nc.vector.memset(ones_col[:, :], 1.0)
        fin_ps = psump.tile([1, 1], F32, tag="g")
        nc.tensor.matmul(fin_ps[:, :], ones_col[:, :], rowsum[:, :],
                         start=True, stop=True)
        fin = singles.tile([1, 1], F32)
        nc.scalar.activation(fin[:, :], fin_ps[:, :], AF.Identity,
                             scale=1.0 / B, bias=cb_fin[:1, :])
        nc.sync.dma_start(out=loss.ap()[:, :], in_=fin[:, :])


def _build(repeat=1, collective=True):
    nc = bacc.Bacc("TRN2", target_bir_lowering=False, debug=False,
                   num_devices=NCORES)
    xt = nc.dram_tensor("xt", [D, B], F8, kind="ExternalInput")
    wt = nc.dram_tensor("wt", [D, CLOC], F8, kind="ExternalInput")
    scl_in = nc.dram_tensor("scl", [128, MT], F32, kind="ExternalInput")
    dneg_in = nc.dram_tensor("dneg", [128, MT], F32, kind="ExternalInput")
    loss = nc.dram_tensor("loss", [1, 1], F32, kind="ExternalOutput")
    with tile.TileContext(nc) as tc:
        for _ in range(repeat):
            _body(nc, tc, xt, wt, scl_in, dneg_in, loss, collective=collective)
    nc.compile()
    return nc


def _get_nc():
    global _NC
    if _NC is None:
        _NC = _build()
    return _NC


def _prep(inputs):
    x = np.asarray(inputs["input"], dtype=np.float32)
    label = np.asarray(inputs["label"]).astype(np.int64)
    w = np.asarray(inputs["weight"], dtype=np.float32)
    wnorm = np.sqrt((w * w).sum(axis=1, keepdims=True, dtype=np.float32))
    wn = w / np.maximum(wnorm, 1e-12)
    xnorm = np.sqrt((x * x).sum(axis=1, dtype=np.float32))
    xs = (S / np.maximum(xnorm, 1e-12))[:, None] * x
    xt8 = np.ascontiguousarray(xs.T.astype(ml_dtypes.float8_e4m3))
    wn8 = wn.astype(ml_dtypes.float8_e4m3)
    # exact label-column fixup terms, [B] -> [128, MT] with b = m*128 + p
    cos_lab = (x[np.arange(B)] * wn[label]).sum(axis=1, dtype=np.float64) \
        / xnorm.astype(np.float64)
    scl = S * cos_lab
    dneg = np.exp(scl - SHIFT) - np.exp(scl - SHIFT - S * MARGIN)
    scl_t = np.ascontiguousarray(
        scl.reshape(MT, 128).T.astype(np.float32))
    dneg_t = np.ascontiguousarray(
        dneg.reshape(MT, 128).T.astype(np.float32))
    in_maps = []
    for k in range(NCORES):
        shard = np.zeros((D, CLOC), dtype=ml_dtypes.float8_e4m3)
        shard[:, :CSHARD] = wn8[k * CSHARD:(k + 1) * CSHARD].T
        in_maps.append({"xt": xt8, "wt": shard, "scl": scl_t, "dneg": dneg_t})
    return in_maps


def kernel(**inputs):
    global LAST_RESULTS
    # this axon client build has no NTFF hook; a stray BASS_TRACE=1 in the
    # environment would crash run_bass_kernel_spmd on an optional import
    _os.environ["BASS_NEVER_TRACE"] = "1"
    nc = _get_nc()
    in_maps = _prep(inputs)
    res = run_bass_kernel_spmd(nc, in_maps, core_ids=list(range(NCORES)))
    LAST_RESULTS = res
    return np.asarray(res.results[0]["loss"][0, 0], dtype=np.float32)
